# revision 1
# baseline (speedup 1.0000x reference)
"""Trainium2 kernel for nn_Attention_33 (9-tile channel-attention, Restormer-style).

Strategy: the computation decomposes into 9 tiles x 4 batch = 36 fully
independent (tile, batch) work items (the attention is per-item; no
cross-item reduction).  We shard the 36 items across the 8 NeuronCores
(5 slots per core, 4 dummy slots) and run the per-item fused block on
each core; the host reassembles the 3x3 tile grid.
"""
import numpy as np

B, C, H, W = 4, 128, 384, 384
HEADS = 8
T = 9
HH, WW = H // 3, W // 3          # 128, 128
N_CORES = 8
SLOTS = 5                        # ceil(36/8)

_jit_cache = {}


def _get_runner():
    if "run" in _jit_cache:
        return _jit_cache["run"]

    import jax
    import jax.numpy as jnp
    from jax import lax

    def _item(x, ln_w, ln_b, qkv_w, qkv_b, dw_w, dw_b, proj_w, proj_b, temp, grw):
        # x: [C, HH, WW] one (tile, batch) item
        c, h, w = x.shape
        res = x
        mu = jnp.mean(x, axis=0, keepdims=True)
        var = jnp.mean((x - mu) ** 2, axis=0, keepdims=True)
        y = (x - mu) / jnp.sqrt(var + 1e-5) * ln_w[:, None, None] + ln_b[:, None, None]
        qkv = jnp.einsum('chw,oc->ohw', y, qkv_w) + qkv_b[:, None, None]
        # depthwise 3x3, padding 1, as 9 shifted multiply-adds (XLA-friendly)
        qp = jnp.pad(qkv, ((0, 0), (1, 1), (1, 1)))
        acc = dw_b[:, None, None]
        for dr in range(3):
            for dc in range(3):
                acc = acc + dw_w[:, 0, dr, dc, None, None] * \
                    lax.dynamic_slice(qp, (0, dr, dc), (3 * c, h, w))
        qkv = acc
        q, k, v = jnp.split(qkv, 3, axis=0)
        heads = lambda t_: t_.reshape(HEADS, c // HEADS, h * w)
        q, k, v = heads(q), heads(k), heads(v)
        q = q / jnp.maximum(jnp.linalg.norm(q, axis=-1, keepdims=True), 1e-12)
        k = k / jnp.maximum(jnp.linalg.norm(k, axis=-1, keepdims=True), 1e-12)
        attn = jnp.einsum('hcn,hdn->hcd', q, k) * temp[:, None, None]
        attn = jax.nn.softmax(attn, axis=-1)
        out = jnp.einsum('hcd,hdn->hcn', attn, v).reshape(c, h, w)
        out = jnp.einsum('chw,oc->ohw', out, proj_w) + proj_b[:, None, None]
        return grw * res + out

    def _shard(xs, ln_w, ln_b, qkv_w, qkv_b, dw_w, dw_b, proj_w, proj_b, temp, grw):
        # xs: [SLOTS, C, HH, WW]; params: [SLOTS, ...]
        return jax.vmap(_item)(xs, ln_w, ln_b, qkv_w, qkv_b, dw_w, dw_b,
                               proj_w, proj_b, temp, grw)

    run = jax.pmap(_shard, axis_name='cores')
    _jit_cache["run"] = run
    return run


def kernel(x, ln_w, ln_b, qkv_w, qkv_b, dw_w, dw_b, proj_w, proj_b,
           temperature, grw):
    run = _get_runner()

    # host-side sharding: [B,C,H,W] -> [T,B,C,HH,WW] (row-major tile order)
    tiles = x.reshape(B, C, 3, HH, 3, WW).transpose(2, 4, 0, 1, 3, 5) \
             .reshape(T, B, C, HH, WW)
    items_x = tiles.reshape(T * B, C, HH, WW)            # item j = (t=j//B, b=j%B)

    t_idx = np.arange(T * B) // B                         # tile index per item
    pad = N_CORES * SLOTS - T * B                         # 4 dummy slots
    t_idx = np.concatenate([t_idx, np.zeros(pad, np.int64)])
    items_x = np.concatenate([items_x, np.zeros((pad, C, HH, WW), items_x.dtype)])

    def sh(p):  # per-item param gather -> [N_CORES, SLOTS, ...]
        g = np.ascontiguousarray(p[t_idx])
        return g.reshape(N_CORES, SLOTS, *p.shape[1:])

    xs = items_x.reshape(N_CORES, SLOTS, C, HH, WW)
    out = run(xs, sh(ln_w), sh(ln_b), sh(qkv_w), sh(qkv_b), sh(dw_w),
              sh(dw_b), sh(proj_w), sh(proj_b), sh(temperature), sh(grw))
    out = np.asarray(out).reshape(N_CORES * SLOTS, C, HH, WW)[:T * B]

    # reassemble 3x3 grid
    out = out.reshape(3, 3, B, C, HH, WW).transpose(2, 3, 0, 4, 1, 5) \
             .reshape(B, C, H, W)
    return out.astype(x.dtype)



# revision 6
# speedup vs baseline: 1.1453x; 1.1453x over previous
"""Trainium2 Bass kernel for nn_Attention_33 (9-tile channel attention).

36 (tile, batch) items sharded across 8 NeuronCores x 5 slots (4 dummy).
Per item [C=128, 128x128 px]: LayerNorm(channel) -> 1x1 qkv conv (PE, bf16)
-> depthwise 3x3 (PE diag-matmul for dc!=0 taps + DVE scalar_tensor_tensor
for dc==0 taps) -> q,k DMA-transposed -> Gram matmul on PE -> blockdiag
softmax (q/k normalization folded into the small [128,128] Gram matrix)
-> attnV + proj on PE -> fused residual.  LN applied as y = a[n]*x + b[n]
with per-pixel a,b from reversed per-chunk stats matmuls, transposed on PE
and partition-broadcast on GPSIMD.
"""
import sys, os
sys.path.insert(0, '/opt/trn_rl_repo')
import numpy as np
import ml_dtypes

B, C, H, W = 4, 128, 384, 384
HEADS = 8
T = 9
HH = WW = 128
N = HH * WW                      # 16384 pixels per item
N_CORES = 8
SLOTS = 5                        # ceil(36/8)
NSLAB = 8
ROWS = 16                        # output rows per slab

_cache = {}


def _build():
    import concourse.bass as bass
    import concourse.tile as tile
    import concourse.mybir as mybir
    from concourse import bacc

    F32, BF16 = mybir.dt.float32, mybir.dt.bfloat16
    AL = mybir.AluOpType
    AF = mybir.ActivationFunctionType

    nc = bacc.Bacc("TRN2", target_bir_lowering=False, debug=False,
                   num_devices=N_CORES)

    def din(name, shape, dt):
        return nc.dram_tensor(name, list(shape), dt, kind="ExternalInput").ap()

    xs_d  = din("xs",  [SLOTS, C, N], F32)
    wg_d  = din("wg",  [SLOTS, C, 3 * C], BF16)      # (qkv_w * ln_w).T  [c, o]
    qb_d  = din("qb",  [SLOTS, C, 3], F32)           # qkv bias per block col
    dwd_d = din("dwd", [SLOTS, C, 27, C], BF16)      # diag mats [r, b*9+tap, c]
    dwc_d = din("dwc", [SLOTS, C, 27], F32)          # per-chan tap scalars
    dwb_d = din("dwb", [SLOTS, C, 3], F32)           # dw bias per block col
    pw_d  = din("pw",  [SLOTS, C, C], BF16)          # proj_w.T [c, p]
    pb_d  = din("pb",  [SLOTS, C, 1], F32)
    tau_d = din("tau", [SLOTS, C, 1], F32)           # temperature per d-chan
    grw_d = din("grw", [SLOTS, C, 1], F32)
    msk_d = din("msk", [C, C], F32)                  # blockdiag mask
    idf_d = din("idf", [C, C], F32)                  # f32 identity
    or_d  = din("onr", [1, C], BF16)                 # ones row
    oc_d  = din("onc", [C, 2], BF16)                 # col0: 1/128, col1: 1.0

    out_d = nc.dram_tensor("out", [SLOTS, C, N], F32, kind="ExternalOutput").ap()

    def slab_rows(s):
        r_lo = max(0, ROWS * s - 1)
        r_hi = min(HH, ROWS * s + ROWS + 1)
        return r_lo, r_hi - r_lo  # start, count (17 or 18)

    with tile.TileContext(nc) as tc:
        import contextlib
        ctx = contextlib.ExitStack()
        pers  = ctx.enter_context(tc.tile_pool(name="pers", bufs=1))
        itemp = ctx.enter_context(tc.tile_pool(name="item", bufs=1))
        parp  = ctx.enter_context(tc.tile_pool(name="par", bufs=1))
        slabp = ctx.enter_context(tc.tile_pool(name="slab", bufs=2))
        slab1 = ctx.enter_context(tc.tile_pool(name="slab1", bufs=1))
        smal  = ctx.enter_context(tc.tile_pool(name="small", bufs=2))
        ph3   = ctx.enter_context(tc.tile_pool(name="ph3", bufs=3))
        ps_ms = ctx.enter_context(tc.tile_pool(name="ps_ms", bufs=2, space="PSUM"))
        ps_qk = ctx.enter_context(tc.tile_pool(name="ps_qk", bufs=3, space="PSUM"))
        ps_cv = ctx.enter_context(tc.tile_pool(name="ps_cv", bufs=2, space="PSUM"))
        ps_g  = ctx.enter_context(tc.tile_pool(name="ps_g", bufs=1, space="PSUM"))

        # shared constants
        msk = pers.tile([C, C], F32);  nc.sync.dma_start(msk[:], msk_d)
        idf = pers.tile([C, C], F32);  nc.sync.dma_start(idf[:], idf_d)
        onr = pers.tile([1, C], BF16); nc.sync.dma_start(onr[:], or_d)
        onc = pers.tile([C, 2], BF16); nc.sync.dma_start(onc[:], oc_d)
        eps5 = pers.tile([C, 1], F32); nc.vector.memset(eps5[:], 1e-5)
        eps24 = pers.tile([C, 1], F32); nc.vector.memset(eps24[:], 1e-24)

        for slot in range(SLOTS):
            # ---- per-item loads ----
            v_fl = itemp.tile([C, HH, WW], BF16, tag="v_fl")
            wg  = parp.tile([C, 3 * C], BF16, tag="wg");  nc.sync.dma_start(wg[:], wg_d[slot])
            qb  = parp.tile([C, 3], F32, tag="qb");   nc.sync.dma_start(qb[:], qb_d[slot])
            dwd = parp.tile([C, 27, C], BF16, tag="dwd"); nc.sync.dma_start(dwd[:], dwd_d[slot])
            dwc = parp.tile([C, 27], F32, tag="dwc"); nc.sync.dma_start(dwc[:], dwc_d[slot])
            dwb = parp.tile([C, 3], F32, tag="dwb");  nc.sync.dma_start(dwb[:], dwb_d[slot])
            pw  = parp.tile([C, C], BF16, tag="pw");  nc.sync.dma_start(pw[:], pw_d[slot])
            tau = parp.tile([C, 1], F32, tag="tau");  nc.sync.dma_start(tau[:], tau_d[slot])
            grw = parp.tile([C, 1], F32, tag="grw");  nc.sync.dma_start(grw[:], grw_d[slot])

            psG = ps_g.tile([C, C], F32, tag="G")
            nq = smal.tile([C, 1], F32, tag="nq"); nc.vector.memset(nq[:], 0.0)
            nk = smal.tile([C, 1], F32, tag="nk"); nc.vector.memset(nk[:], 0.0)

            for s in range(NSLAB):
                r_lo, R = slab_rows(s)
                n_in = R * WW
                r_off = 0 if s == 0 else 1

                xsl = slabp.tile([C, 18, WW], BF16, tag="xsl")
                nc.gpsimd.dma_start(xsl[:, :R, :],
                                    xs_d[slot][:, r_lo * WW:(r_lo + R) * WW])
                xfl = xsl[:, :R, :].rearrange("p a b -> p (a b)")

                # ---- stats: per-chunk reversed matmuls (mu, E2 per pixel) ----
                x2 = slab1.tile([C, 18 * WW], BF16, tag="x2")
                nc.scalar.activation(x2[:, :n_in], xfl, AF.Square)
                pst = ps_ms.tile([C, 40], F32, tag="ms")
                for r in range(R):
                    nc.tensor.matmul(pst[:, 2 * r:2 * r + 1], xsl[:, r, :],
                                     onc[:, 0:1], start=True, stop=True)
                    nc.tensor.matmul(pst[:, 2 * r + 1:2 * r + 2],
                                     x2[:, r * WW:(r + 1) * WW],
                                     onc[:, 0:1], start=True, stop=True)
                st = smal.tile([C, 40], F32, tag="st")
                nc.vector.tensor_copy(st[:, :2 * R], pst[:, :2 * R])
                mu = st[:, 0:2 * R:2]
                e2 = st[:, 1:2 * R:2]
                ab = smal.tile([C, 2, 18], F32, tag="ab")      # a rows | b rows
                nc.vector.scalar_tensor_tensor(ab[:, 1, :R], mu, -1.0, mu,
                                               op0=AL.mult, op1=AL.mult)   # -mu^2
                nc.vector.tensor_tensor(ab[:, 0, :R], e2, ab[:, 1, :R], op=AL.add)
                nc.scalar.activation(ab[:, 0, :R], ab[:, 0, :R], AF.Sqrt, bias=eps5[:])
                nc.vector.reciprocal(ab[:, 0, :R], ab[:, 0, :R])           # a = rstd
                nc.vector.scalar_tensor_tensor(ab[:, 1, :R], mu, -1.0, ab[:, 0, :R],
                                               op0=AL.mult, op1=AL.mult)   # b = -mu*a
                pab = ps_ms.tile([36, C], F32, tag="ms")
                nc.tensor.transpose(pab[:], ab.rearrange("p a b -> p (a b)"), idf[:])
                abT = smal.tile([36, C], BF16, tag="abT")
                nc.scalar.copy(abT[:], pab[:])
                arow = slab1.tile([1, 18 * WW], BF16, tag="arow")
                brow = slab1.tile([1, 18 * WW], BF16, tag="brow")
                nc.sync.dma_start(arow[:, :n_in], abT[0:R, :])
                nc.sync.dma_start(brow[:, :n_in], abT[18:18 + R, :])
                aB = slab1.tile([C, 18 * WW], BF16, tag="aB")
                bB = slab1.tile([C, 18 * WW], BF16, tag="bB")
                nc.gpsimd.partition_broadcast(aB[:, :n_in], arow[:, :n_in])
                nc.gpsimd.partition_broadcast(bB[:, :n_in], brow[:, :n_in])

                # ---- y = a*x + b ----
                y = slabp.tile([C, 18 * WW], BF16, tag="y")
                nc.gpsimd.tensor_tensor(y[:, :n_in], xfl, aB[:, :n_in], op=AL.mult)
                nc.gpsimd.tensor_tensor(y[:, :n_in], y[:, :n_in], bB[:, :n_in],
                                        op=AL.add)

                # ---- qkv 1x1 conv ----
                qkv = [slabp.tile([C, 18, WW], BF16, name=f"qkv{b}",
                                  tag=f"qkv{b}") for b in range(3)]
                ntiles = [(i * 512, min(512, n_in - i * 512))
                          for i in range((n_in + 511) // 512)]
                for b in range(3):
                    qfl = qkv[b].rearrange("p a b -> p (a b)")
                    for ti, (o, ln) in enumerate(ntiles):
                        pq = ps_qk.tile([C, 512], F32, tag="pq")
                        nc.tensor.matmul(pq[:, :ln], wg[:, b * C:(b + 1) * C],
                                         y[:, o:o + ln], start=True, stop=True)
                        if ti % 2 == 0:
                            nc.scalar.activation(qfl[:, o:o + ln], pq[:, :ln],
                                                 AF.Identity, bias=qb[:, b:b + 1])
                        else:
                            nc.vector.tensor_scalar_add(qfl[:, o:o + ln],
                                                        pq[:, :ln], qb[:, b:b + 1])

                # ---- depthwise 3x3 ----
                q_cv = slabp.tile([C, ROWS, WW], BF16, tag="q_cv")
                k_cv = slabp.tile([C, ROWS, WW], BF16, tag="k_cv")
                for b in range(3):
                    dest = (q_cv, k_cv, None)[b]
                    dst = dest if dest is not None else \
                        v_fl[:, ROWS * s:ROWS * (s + 1), :]
                    acc = slabp.tile([C, ROWS, WW], BF16, tag="acc")
                    # DVE: the three dc == 0 taps
                    ctr = b * 9 + 4
                    nc.vector.tensor_scalar(
                        acc[:], qkv[b][:, r_off:r_off + ROWS, :],
                        dwc[:, ctr:ctr + 1], dwb[:, b:b + 1],
                        op0=AL.mult, op1=AL.add)
                    for dr in (-1, 1):
                        j0 = 1 if (s == 0 and dr == -1) else 0
                        j1 = (ROWS - 1) if (s == NSLAB - 1 and dr == 1) else ROWS
                        tap = b * 9 + (dr + 1) * 3 + 1
                        nc.vector.scalar_tensor_tensor(
                            acc[:, j0:j1, :],
                            qkv[b][:, j0 + r_off + dr:j1 + r_off + dr, :],
                            dwc[:, tap:tap + 1], acc[:, j0:j1, :],
                            op0=AL.mult, op1=AL.add)
                    # PE: the six dc != 0 taps, quarter-slab psum tiles
                    for qt in range(4):
                        q0 = 4 * qt
                        pcv = ps_cv.tile([C, 4, WW], F32, tag="pcv")
                        first = True
                        mms = []
                        for dr in (-1, 0, 1):
                            j0 = max(q0, 1 if (s == 0 and dr == -1) else 0)
                            j1 = min(q0 + 4, (ROWS - 1)
                                     if (s == NSLAB - 1 and dr == 1) else ROWS)
                            if j0 >= j1:
                                continue
                            for dc in (-1, 1):
                                c0, c1 = (1, WW) if dc == -1 else (0, WW - 1)
                                mms.append((j0, j1, dr, dc, c0, c1))
                        for mi, (j0, j1, dr, dc, c0, c1) in enumerate(mms):
                            tap = b * 9 + (dr + 1) * 3 + (dc + 1)
                            nc.tensor.matmul(
                                pcv[:, j0 - q0:j1 - q0, c0:c1],
                                dwd[:, tap, :],
                                qkv[b][:, j0 + r_off + dr:j1 + r_off + dr,
                                       c0 + dc:c1 + dc],
                                start=(mi == 0), stop=(mi == len(mms) - 1),
                                skip_group_check=True)
                        nc.vector.scalar_tensor_tensor(
                            dst[:, q0:q0 + 4, :], acc[:, q0:q0 + 4, :], 1.0,
                            pcv[:], op0=AL.mult, op1=AL.add)

                # ---- transposes, Gram, norms ----
                qT = slabp.tile([C, ROWS, WW], BF16, tag="qT")
                kT = slabp.tile([C, ROWS, WW], BF16, tag="kT")
                nc.sync.dma_start(qT[:], q_cv.rearrange("p a b -> p (a b)"),
                                  transpose=True)
                nc.sync.dma_start(kT[:], k_cv.rearrange("p a b -> p (a b)"),
                                  transpose=True)
                for j in range(ROWS):
                    nc.tensor.matmul(psG[:], kT[:, j, :], qT[:, j, :],
                                     start=(s == 0 and j == 0),
                                     stop=(s == NSLAB - 1 and j == ROWS - 1))
                sq = slab1.tile([C, ROWS * WW], BF16, tag="sq")
                nqp = smal.tile([C, 1], F32, tag="nqp")
                nkp = smal.tile([C, 1], F32, tag="nkp")
                nc.scalar.activation(sq[:], q_cv.rearrange("p a b -> p (a b)"),
                                     AF.Square, accum_out=nqp[:])
                nc.vector.tensor_tensor(nq[:], nq[:], nqp[:], op=AL.add)
                nc.scalar.activation(sq[:], k_cv.rearrange("p a b -> p (a b)"),
                                     AF.Square, accum_out=nkp[:])
                nc.vector.tensor_tensor(nk[:], nk[:], nkp[:], op=AL.add)

            # ---- softmax prep (small) ----
            rk = smal.tile([C, 1], F32, tag="rk")
            nc.scalar.activation(rk[:], nk[:], AF.Sqrt, bias=eps24[:])
            nc.vector.reciprocal(rk[:], rk[:])
            nc.vector.tensor_tensor(rk[:], rk[:], tau[:], op=AL.mult)   # rk*tau
            rq = smal.tile([C, 1], F32, tag="rq")
            nc.scalar.activation(rq[:], nq[:], AF.Sqrt, bias=eps24[:])
            nc.vector.reciprocal(rq[:], rq[:])
            prqT = ps_ms.tile([1, C], F32, tag="ms")
            nc.tensor.transpose(prqT[:], rq[:], idf[:])
            rqT = smal.tile([1, C], BF16, tag="rqT")
            nc.scalar.copy(rqT[:], prqT[:])
            prqB = ps_ms.tile([C, C], F32, tag="ms")
            nc.tensor.matmul(prqB[:], onr[:], rqT[:], start=True, stop=True)

            lg = smal.tile([C, C], F32, tag="lg")
            nc.scalar.activation(lg[:], psG[:], AF.Copy, scale=rk[:])
            nc.vector.tensor_tensor(lg[:], lg[:], prqB[:], op=AL.mult)
            nc.scalar.activation(lg[:], lg[:], AF.Exp)
            expm = smal.tile([C, C], BF16, tag="expm")
            nc.vector.tensor_tensor(expm[:], lg[:], msk[:], op=AL.mult)
            pcs = ps_ms.tile([C, 1], F32, tag="ms")
            nc.tensor.matmul(pcs[:], expm[:], onc[:, 1:2], start=True, stop=True)
            inv = smal.tile([C, 1], F32, tag="inv")
            nc.vector.reciprocal(inv[:], pcs[:])

            # ---- attnV + proj + residual ----
            vfl = v_fl.rearrange("p a b -> p (a b)")
            for i in range(N // 512):
                sl = slice(i * 512, (i + 1) * 512)
                xb3 = ph3.tile([C, 512], BF16, tag="xb3")
                nc.gpsimd.dma_start(xb3[:], xs_d[slot][:, sl])
                pav = ps_qk.tile([C, 512], F32, tag="pq")
                nc.tensor.matmul(pav[:], expm[:], vfl[:, sl], start=True, stop=True)
                o2 = ph3.tile([C, 512], BF16, tag="o2")
                nc.scalar.activation(o2[:], pav[:], AF.Copy, scale=inv[:])
                ppj = ps_qk.tile([C, 512], F32, tag="pq")
                nc.tensor.matmul(ppj[:], pw[:], o2[:], start=True, stop=True)
                of = ph3.tile([C, 512], F32, tag="of")
                nc.vector.scalar_tensor_tensor(of[:], xb3[:], grw[:], ppj[:],
                                               op0=AL.mult, op1=AL.add)
                nc.sync.dma_start(out_d[slot][:, sl], of[:])
        ctx.close()

    nc.compile()
    return nc


def _get_runner():
    if "run" in _cache:
        return _cache["run"]
    import jax
    from jax.sharding import Mesh, PartitionSpec
    from jax.experimental.shard_map import shard_map
    import concourse.mybir as mybir
    from concourse.bass2jax import (_bass_exec_p, install_neuronx_cc_hook,
                                    partition_id_tensor)

    nc = _build()
    install_neuronx_cc_hook()

    part_name = nc.partition_id_tensor.name if nc.partition_id_tensor else None
    in_names, out_names, out_avals, zero_outs = [], [], [], []
    for alloc in nc.m.functions[0].allocations:
        if not isinstance(alloc, mybir.MemoryLocationSet):
            continue
        name = alloc.memorylocations[0].name
        if alloc.kind == "ExternalInput":
            if name != part_name:
                in_names.append(name)
        elif alloc.kind == "ExternalOutput":
            out_names.append(name)
            shape = tuple(alloc.tensor_shape)
            dtype = mybir.dt.np(alloc.dtype)
            out_avals.append(jax.core.ShapedArray(shape, dtype))
            zero_outs.append(np.zeros(shape, dtype))
    n_params = len(in_names)
    all_names = in_names + out_names
    if part_name is not None:
        all_names = all_names + [part_name]

    def _body(*args):
        operands = list(args)
        if part_name is not None:
            operands.append(partition_id_tensor())
        outs = _bass_exec_p.bind(
            *operands, out_avals=tuple(out_avals), in_names=tuple(all_names),
            out_names=tuple(out_names), lowering_input_output_aliases=(),
            sim_require_finite=False, sim_require_nnan=False, nc=nc)
        return tuple(outs)

    devices = jax.devices()[:N_CORES]
    mesh = Mesh(np.asarray(devices), ("core",))
    nio = n_params + len(out_names)
    sharded = jax.jit(shard_map(_body, mesh=mesh,
                                in_specs=(PartitionSpec("core"),) * nio,
                                out_specs=(PartitionSpec("core"),) * len(out_names),
                                check_rep=False),
                      keep_unused=True)
    run = dict(fn=sharded, in_names=in_names, out_names=out_names,
               zero_outs=zero_outs, nc=nc)
    _cache["run"] = run
    return run


def _prep_inputs(x, ln_w, ln_b, qkv_w, qkv_b, dw_w, dw_b, proj_w, proj_b,
                 temperature, grw):
    """Host-side shard + param folding. Returns concat arrays per input name."""
    bf16 = ml_dtypes.bfloat16
    f32 = np.float32
    tiles = x.reshape(B, C, 3, HH, 3, WW).transpose(2, 4, 0, 1, 3, 5) \
             .reshape(T * B, C, N).astype(f32)
    n_items = N_CORES * SLOTS
    xs = np.zeros((n_items, C, N), f32)
    xs[:T * B] = tiles
    t_idx = np.concatenate([np.arange(T * B) // B,
                            np.zeros(n_items - T * B, np.int64)])

    wg_t = (qkv_w * ln_w[:, None, :]).transpose(0, 2, 1).astype(bf16)   # [T,c,3C]
    qb_t = (qkv_b + np.einsum('toc,tc->to', qkv_w, ln_b)) \
        .reshape(T, 3, C).transpose(0, 2, 1).astype(f32)                # [T,c,3]
    dw = dw_w[:, :, 0]                                                  # [T,3C,3,3]
    dwc_t = dw.reshape(T, 3, C, 9).transpose(0, 2, 1, 3) \
              .reshape(T, C, 27).astype(f32)
    dwd_t = np.zeros((T, C, 27, C), bf16)
    r = np.arange(C)
    for bb in range(3):
        for tap in range(9):
            dwd_t[:, r, bb * 9 + tap, r] = dw[:, bb * C + r, tap // 3, tap % 3] \
                .astype(bf16)
    dwb_t = dw_b.reshape(T, 3, C).transpose(0, 2, 1).astype(f32)
    pw_t = proj_w.transpose(0, 2, 1).astype(bf16)
    pb_t = proj_b[:, :, None].astype(f32)
    tau_t = np.repeat(temperature, C // HEADS, axis=1)[:, :, None].astype(f32)
    grw_t = np.broadcast_to(grw[:, None, None], (T, C, 1)).astype(f32)

    def gat(p):
        return np.ascontiguousarray(p[t_idx]).reshape(N_CORES, SLOTS,
                                                      *p.shape[1:])

    per = {
        "xs": xs.reshape(N_CORES, SLOTS, C, N),
        "wg": gat(wg_t), "qb": gat(qb_t), "dwd": gat(dwd_t), "dwc": gat(dwc_t),
        "dwb": gat(dwb_t), "pw": gat(pw_t), "pb": gat(pb_t), "tau": gat(tau_t),
        "grw": gat(grw_t),
    }
    msk = np.kron(np.eye(HEADS, dtype=f32),
                  np.ones((C // HEADS, C // HEADS), f32))
    shared = {
        "msk": msk, "idf": np.eye(C, dtype=f32),
        "onr": np.ones((1, C), bf16),
        "onc": np.stack([np.full(C, 1.0 / C), np.ones(C)], 1).astype(bf16),
    }
    concat = {}
    for k, v in per.items():
        concat[k] = v.reshape(N_CORES * v.shape[1], *v.shape[2:])
    for k, v in shared.items():
        concat[k] = np.concatenate([v] * N_CORES, axis=0)
    return concat


def _run_device(concat):
    run = _get_runner()
    args = [concat[n] for n in run["in_names"]]
    args += [np.concatenate([z] * N_CORES, 0) for z in run["zero_outs"]]
    return run["fn"](*args)


def kernel(x, ln_w, ln_b, qkv_w, qkv_b, dw_w, dw_b, proj_w, proj_b,
           temperature, grw):
    concat = _prep_inputs(np.asarray(x), *[np.asarray(a) for a in
              (ln_w, ln_b, qkv_w, qkv_b, dw_w, dw_b, proj_w, proj_b,
               temperature, grw)])
    outs = _run_device(concat)
    run = _cache["run"]
    out = np.asarray(outs[run["out_names"].index("out")])
    out = out.reshape(N_CORES * SLOTS, C, N)[:T * B]
    out = out.reshape(3, 3, B, C, HH, WW).transpose(2, 3, 0, 4, 1, 5) \
             .reshape(B, C, H, W)
    return out.astype(np.float32)
